# revision 27
# baseline (speedup 1.0000x reference)
"""Causal self-attention (RMS-normed QK + partial RoPE + lambda-blended V)
for Trainium2, tensor-parallel over heads across 8 NeuronCores.

Per core: 2 heads. Host pre-transposes x to bf16 x^T (kills the on-device
cast + DRAM round-trip + DMA-transpose stage) and lays every constant out
partition-major so each DMA is contiguous per partition.

Device pipeline per 512-token block:
  QKV matmuls from resident x^T tiles (bf16, fp32 psum)
  -> RMS stats on DVE (sq + grouped reduce), rstd = exp(-.5 ln) on ACT
  -> normalize (tensor_scalar, 4x mode) + batched RoPE (DVE)
  -> q,k head-tiles transposed via DMA xbar (SBUF->SBUF) into [dh, t]
  -> flash-style causal attention: scores^T on PE, exp on ACT (fp16 probs),
     fp16 l accumulation, PV accumulation in psum
  -> l all-reduced across partitions on GPSIMD, reciprocal on DVE,
     folded into o during psum evacuation (one multiply per head-block)
  -> output projection (2-head psum accumulation), fp32 out.
Host: shards weights per core, sums the 8 partial outputs.
"""
import sys
sys.path.insert(0, "/opt/trn_rl_repo")

import math
import numpy as np
import ml_dtypes

import concourse.bass as bass
import concourse.tile as tile
from concourse import bacc, bass_isa, mybir
from concourse.masks import make_identity

# All ACT functions this kernel uses (exp, ln, square, copy, identity) live in
# the single table set `natural_log_exp_and_others`. The default table-load
# pass picks the FIRST set containing each function (exp -> exp_and_others,
# ln -> natural_log), which thrashes two table sets all kernel long (64 x
# ~1.3us reloads). Restrict every other set's advertised contents so the pass
# lands on the combined set once. Set IDs are positional, so entries are
# filtered in place rather than reordered.
_orig_get_act_tables = bacc.get_activation_tables
_KEEP = {"natural_log_exp_and_others"}


def _patched_get_act_tables(arch):
    tables = _orig_get_act_tables(arch)
    combined = tables.get("natural_log_exp_and_others")
    if combined is None:
        return tables
    out = {}
    for name, fns in tables.items():
        out[name] = fns if name in _KEEP else (fns - combined)
    return out


bacc.get_activation_tables = _patched_get_act_tables

bf16 = ml_dtypes.bfloat16
F32 = mybir.dt.float32
F16 = mybir.dt.float16
BF = mybir.dt.bfloat16
AF = mybir.ActivationFunctionType
ALU = mybir.AluOpType

D = 2048          # model dim
NH = 16           # total heads
DH = 128          # head dim
NCORES = 8
HPC = NH // NCORES          # heads per core = 2
DLOC = HPC * DH             # local hdim = 256
EPS = 1e-6
TB = 512                    # t-block size
SQRT_DH = math.sqrt(DH)

_BUILD_CACHE = {}


def _build(T):
    """Build the per-core Bass program (same program on all cores)."""
    NTB = T // TB
    nc = bacc.Bacc("TRN2", target_bir_lowering=False)

    xt_in = nc.dram_tensor("xt", [D, T], BF, kind="ExternalInput")
    wq_in = nc.dram_tensor("wqkv", [128, D // 128, 3 * DLOC], BF, kind="ExternalInput")
    wp_in = nc.dram_tensor("wproj", [128, HPC, D], BF, kind="ExternalInput")
    ve_in = nc.dram_tensor("ve", [NTB, 128, 4, DLOC], F16, kind="ExternalInput")
    cos_in = nc.dram_tensor("cos", [128, T // 128, 32], F32, kind="ExternalInput")
    sin_in = nc.dram_tensor("sin", [128, T // 128, 32], F32, kind="ExternalInput")
    mask_in = nc.dram_tensor("mask", [128, 4, TB], F16, kind="ExternalInput")
    out_d = nc.dram_tensor("out", [T, D], BF, kind="ExternalOutput")

    with tile.TileContext(nc) as tc:
        with (
            tc.tile_pool(name="const", bufs=1) as const,
            tc.tile_pool(name="res", bufs=1) as res,
            tc.tile_pool(name="xt", bufs=3) as xtp,
            tc.tile_pool(name="work", bufs=2) as work,
            tc.tile_pool(name="att", bufs=3) as att,
            tc.tile_pool(name="lac", bufs=2) as lac,
            tc.tile_pool(name="prj", bufs=2) as prj,
            tc.tile_pool(name="psQ", bufs=1, space="PSUM") as psQ,
            tc.tile_pool(name="psS", bufs=3, space="PSUM") as psS,
            tc.tile_pool(name="psO", bufs=2, space="PSUM") as psO,
            tc.tile_pool(name="psT", bufs=1, space="PSUM") as psT,
        ):
            # ---------------- constants (all partition-major, contiguous) ----
            # wqkv streams per-dc so QKV(0) can start after the first chunks
            wq_sb = const.tile([128, D // 128, 3 * DLOC], BF, tag="wq")
            for dc in range(D // 128):
                nc.sync.dma_start(wq_sb[:, dc, :], wq_in[:, dc, :])
            wp_sb = const.tile([128, HPC, D], BF, tag="wp")
            cos_sb = const.tile([128, T // 128, 32], F32, tag="cos")
            sin_sb = const.tile([128, T // 128, 32], F32, tag="sin")
            mask_sb = const.tile([128, 4, TB], F16, tag="mask")
            lnbias = const.tile([128, 1], F32, tag="lnbias")
            nc.vector.memset(lnbias[:], float(EPS * SQRT_DH))
            ident = const.tile([128, 128], BF, tag="ident")
            make_identity(nc, ident[:])

            # ---------------- resident per-block tensors ----------------
            qT = [res.tile([128, HPC, TB], BF, tag=f"qT{i}", name=f"qT{i}") for i in range(NTB)]
            kT = [res.tile([128, HPC, TB], BF, tag=f"kT{i}", name=f"kT{i}") for i in range(NTB)]
            vB = [res.tile([128, 4, DLOC], F16, tag=f"v{i}", name=f"v{i}") for i in range(NTB)]

            # x^T / ve tile loads, issued one block ahead so the sync DMA
            # FIFO serves them before the rope-gated transposes of the
            # current block (head-of-line blocking otherwise stalls the
            # next block's QKV matmuls).
            xts, vebs = [None] * NTB, [None] * NTB

            def load_block(ti):
                t0 = ti * TB
                xts[ti] = xtp.tile([128, D // 128, TB], BF, tag="xt", name=f"xt{ti}")
                for dc in range(D // 128):
                    nc.sync.dma_start(xts[ti][:, dc, :], xt_in[dc * 128:(dc + 1) * 128, t0:t0 + TB])
                vebs[ti] = xtp.tile([128, 4, DLOC], F16, tag="veb", name=f"veb{ti}")
                nc.gpsimd.dma_start(vebs[ti][:], ve_in[ti])

            def qkv_stage(ti):
                """Generator: yields after each ~1.3us unit of PE work so the
                caller can interleave these matmuls between attention chunks
                (keeps the PE fed while ACT serializes on exp)."""
                xt, veb = xts[ti], vebs[ti]
                for sub in range(4):
                    tg = ti * 4 + sub
                    qkv_ps = psQ.tile([128, 1024], F32, tag="qkv")
                    ndc = D // 128
                    for dc in range(ndc):
                        lhsT = xt[:, dc, sub * 128:(sub + 1) * 128]
                        st, sp = dc == 0, dc == ndc - 1
                        nc.tensor.matmul(qkv_ps[:, 0:512], lhsT, wq_sb[:, dc, 0:512], start=st, stop=sp)
                        nc.tensor.matmul(qkv_ps[:, 512:768], lhsT, wq_sb[:, dc, 512:768], start=st, stop=sp)
                        if dc % 4 == 3:
                            yield
                    # v (lambda0 pre-folded in weights; lambda1*ve added below)
                    nc.vector.tensor_copy(vB[ti][:, sub, :], qkv_ps[:, 512:768])
                    # raw q|k to sbuf bf16 (frees psum fast; enables 2x/4x DVE)
                    qraw = work.tile([128, 512], BF, tag="qraw")
                    nc.vector.tensor_copy(qraw[:], qkv_ps[:, 0:512])
                    # rms statistics: sumsq per head for q and k (DVE)
                    sq = work.tile([128, 512], BF, tag="sq")
                    nc.vector.tensor_mul(sq[:], qraw[:], qraw[:])
                    ssq = work.tile([128, 4], F32, tag="ssq")
                    nc.vector.tensor_reduce(
                        ssq[:, :, None], sq[:].rearrange("p (g d) -> p g d", g=4),
                        axis=mybir.AxisListType.X, op=ALU.add)
                    # rstd' = (sqrt(DH)*(ms+eps))^-1/2 = exp(-0.5*ln(...)); folds the
                    # 1/sqrt(DH) score scale (split as DH^-0.25 into q and k each)
                    rstd = work.tile([128, 4], F32, tag="rstd")
                    nc.scalar.activation(rstd[:], ssq[:], AF.Ln,
                                         scale=float(SQRT_DH / DH), bias=lnbias[:])
                    nc.scalar.activation(rstd[:], rstd[:], AF.Exp, scale=-0.5)
                    # normalize + rope (batched over the 4 head-groups)
                    qn = work.tile([128, 4, 128], BF, tag="qn")
                    for g in range(4):
                        nc.vector.tensor_scalar_mul(qn[:, g, :], qraw[:, g * 128:(g + 1) * 128], rstd[:, g:g + 1])
                    cosb = cos_sb[:, tg, :][:, None, :].broadcast_to([128, 4, 32])
                    sinb = sin_sb[:, tg, :][:, None, :].broadcast_to([128, 4, 32])
                    x1 = qn[:, :, 0:32]
                    x2 = qn[:, :, 64:96]
                    r1 = work.tile([128, 4, 32], BF, tag="r1")
                    r2 = work.tile([128, 4, 32], BF, tag="r2")
                    r3 = work.tile([128, 4, 32], BF, tag="r3")
                    r4 = work.tile([128, 4, 32], BF, tag="r4")
                    nc.vector.tensor_mul(r1[:], x1, cosb)
                    nc.vector.tensor_mul(r2[:], x2, sinb)
                    nc.vector.tensor_mul(r3[:], x1, sinb)
                    nc.vector.tensor_mul(r4[:], x2, cosb)
                    nc.vector.tensor_add(x1, r1[:], r2[:])
                    nc.vector.tensor_sub(x2, r4[:], r3[:])
                    # transpose q,k head-tiles into resident [dh, t] on PE
                    # (the DMA xbar path costs ~1.25us per 128x128 tile and
                    # saturates the sync DMA queue)
                    for g in range(4):
                        tp = psT.tile([128, 128], BF, tag="tp")
                        nc.tensor.transpose(tp[:], qn[:, g, :], ident[:])
                        dst = qT[ti] if g < 2 else kT[ti]
                        nc.vector.tensor_copy(dst[:, g % 2, sub * 128:(sub + 1) * 128], tp[:])
                    yield
                # blend lambda1*ve into v
                nc.vector.tensor_add(vB[ti][:], vB[ti][:], veb[:])

            def proj_stage(ti):
                """Generator: one (sub, dn) projection tile per unit.
                Out partials are bf16: 8 per-core partials summed in fp32 on
                the host keep ~0.3% error while halving evac + DMA cost."""
                t0 = ti * TB
                oB = oBs[ti]
                for sub in range(4):
                    out_sb = prj.tile([128, D], BF, tag="outsb")
                    for dn in range(D // 512):
                        pr = psS.tile([128, 512], F32, tag="sc")
                        nc.tensor.matmul(pr[:], oB[:, 0, sub * 128:(sub + 1) * 128],
                                         wp_sb[:, 0, dn * 512:(dn + 1) * 512], start=True, stop=False)
                        nc.tensor.matmul(pr[:], oB[:, 1, sub * 128:(sub + 1) * 128],
                                         wp_sb[:, 1, dn * 512:(dn + 1) * 512], start=False, stop=True)
                        if dn % 2 == 0:
                            nc.vector.tensor_copy(out_sb[:, dn * 512:(dn + 1) * 512], pr[:])
                        else:
                            nc.scalar.copy(out_sb[:, dn * 512:(dn + 1) * 512], pr[:])
                        yield
                    nc.sync.dma_start(out_d[t0 + sub * 128: t0 + (sub + 1) * 128, :], out_sb[:])

            QKV_UNITS = 4 * 5   # 4 subs x (4 matmul units + 1 epilogue)
            PROJ_UNITS = 16

            def attn_stage(ti, background, bg_units):
                """Attention for block ti; pumps `background` generators
                (QKV of ti+1, projection of ti-1) between chunks so the PE
                always has matmul work while ACT serializes on exp."""
                l_acc = lac.tile([128, HPC, TB], F16, tag="lacc")
                lraw = lac.tile([128, HPC, TB], F32, tag="lraw")
                linv = lac.tile([128, HPC, TB], F32, tag="linv")
                oBs[ti] = prj.tile([128, HPC, TB], BF, tag="o", name=f"oB{ti}")
                oB = oBs[ti]
                ns = (ti + 1) * 4
                nchunks = HPC * ns
                pumped = 0

                def pump(target):
                    nonlocal pumped
                    while pumped < target and background:
                        try:
                            next(background[0])
                            pumped += 1
                        except StopIteration:
                            background.pop(0)

                o_ps = [None, None]
                c = 0
                for h in range(HPC):
                    o_ps[h] = psO.tile([128, TB], F32, tag="o", name=f"ops{h}")
                    for sj in range(ns):
                        blk, sb_ = sj // 4, sj % 4
                        sc_ps = psS.tile([128, 512], F32, tag="sc")
                        nc.tensor.matmul(
                            sc_ps[:], kT[blk][:, h, sb_ * 128:(sb_ + 1) * 128],
                            qT[ti][:, h, :], start=True, stop=True)
                        probs = att.tile([128, 512], F16, tag="probs")
                        nc.scalar.activation(probs[:], sc_ps[:], AF.Exp)
                        if blk == ti:  # diagonal block: causal mask
                            nc.vector.tensor_mul(probs[:], probs[:], mask_sb[:, sb_, :])
                        if sj == 0:
                            nc.vector.tensor_copy(l_acc[:, h, :], probs[:])
                        else:
                            nc.vector.tensor_add(l_acc[:, h, :], l_acc[:, h, :], probs[:])
                        nc.tensor.matmul(
                            o_ps[h][:], vB[blk][:, sb_, h * 128:(h + 1) * 128],
                            probs[:], start=(sj == 0), stop=(sj == ns - 1))
                        c += 1
                        pump(bg_units * c // nchunks)
                    # l: all-reduce across partitions (GPSIMD), fast 1/x, fold
                    # into o during psum evacuation. Per head so head 0's
                    # epilogue overlaps head 1's attention.
                    nc.gpsimd.partition_all_reduce(
                        lraw[:, h, :], l_acc[:, h, :], channels=128,
                        reduce_op=bass_isa.ReduceOp.add)
                    nc.vector.reciprocal_approx_fast(linv[:, h, :], lraw[:, h, :])
                    nc.vector.tensor_mul(oB[:, h, :], o_ps[h][:], linv[:, h, :])
                # fully drain the background generators, INCLUDING their
                # post-final-yield epilogues (ve blend, last out-store)
                for g in background:
                    for _ in g:
                        pass

            oBs = [None] * NTB
            load_block(0)
            # remaining constants, behind the block-0 critical loads
            nc.sync.dma_start(cos_sb[:], cos_in[:])
            nc.sync.dma_start(sin_sb[:], sin_in[:])
            nc.sync.dma_start(mask_sb[:], mask_in[:])
            nc.sync.dma_start(wp_sb[:], wp_in[:])
            load_block(1)
            for _ in qkv_stage(0):
                pass
            for ti in range(NTB):
                if ti + 2 < NTB:
                    load_block(ti + 2)
                background, bg_units = [], 0
                if ti + 1 < NTB:
                    background.append(qkv_stage(ti + 1))
                    bg_units += QKV_UNITS
                if ti >= 1:
                    background.append(proj_stage(ti - 1))
                    bg_units += PROJ_UNITS
                attn_stage(ti, background, bg_units)
            for _ in proj_stage(NTB - 1):
                pass
    return nc


def _host_prep(x, ve, lambdas, qkv_w, proj_w, T):
    """Build the 8 per-core input maps (sharding + constant tables)."""
    x = np.asarray(x, np.float32).reshape(T, D)
    ve = np.asarray(ve, np.float32).reshape(T, NH * DH)
    lam = np.asarray(lambdas, np.float32)
    qkv_w = np.asarray(qkv_w, np.float32)
    proj_w = np.asarray(proj_w, np.float32)

    xt = np.ascontiguousarray(x.T).astype(bf16)  # [D, T]

    quarter = DH // 4
    ang = (1.0 / 1024.0) ** np.linspace(0.0, 1.0, quarter, dtype=np.float32)
    theta = np.arange(T, dtype=np.float32)[:, None] * ang[None, :]   # [T, 32]
    cos_t = np.ascontiguousarray(
        np.cos(theta).reshape(T // 128, 128, 32).transpose(1, 0, 2))
    sin_t = np.ascontiguousarray(
        np.sin(theta).reshape(T // 128, 128, 32).transpose(1, 0, 2))

    s_l = np.arange(128)[:, None]
    t_l = np.arange(TB)[None, :]
    mask = np.stack([(t_l >= s_l + 128 * j) for j in range(4)], axis=1).astype(np.float16)

    in_maps = []
    for c in range(NCORES):
        sl = slice(c * DLOC, (c + 1) * DLOC)
        wqkv = np.concatenate(
            [qkv_w[0, sl].T, qkv_w[1, sl].T, lam[0] * qkv_w[2, sl].T], axis=1)  # [D, 768]
        wqkv_pm = np.ascontiguousarray(
            wqkv.reshape(D // 128, 128, 3 * DLOC).transpose(1, 0, 2)).astype(bf16)
        wp = np.ascontiguousarray(proj_w[:, sl].T)  # [DLOC, D]
        wp_pm = np.ascontiguousarray(
            wp.reshape(HPC, 128, D).transpose(1, 0, 2)).astype(bf16)
        ve_pm = np.ascontiguousarray(
            (lam[1] * ve[:, sl]).reshape(T // TB, 4, 128, DLOC).transpose(0, 2, 1, 3)
        ).astype(np.float16)
        in_maps.append({
            "xt": xt,
            "wqkv": wqkv_pm,
            "wproj": wp_pm,
            "ve": ve_pm,
            "cos": cos_t, "sin": sin_t, "mask": mask,
        })
    return in_maps


def kernel(x, ve, lambdas, qkv_w, proj_w):
    B, T, _ = x.shape
    in_maps = _host_prep(x, ve, lambdas, qkv_w, proj_w, T)
    if T not in _BUILD_CACHE:
        nc = _build(T)
        nc.compile()
        _BUILD_CACHE[T] = nc
    nc = _BUILD_CACHE[T]

    from concourse.bass_utils import run_bass_kernel_spmd
    res = run_bass_kernel_spmd(nc, in_maps, core_ids=list(range(NCORES)))
    out = np.zeros((T, D), np.float32)
    for c in range(NCORES):
        out += res.results[c]["out"].astype(np.float32)
    return out.reshape(B, T, D)
